# revision 1
# baseline (speedup 1.0000x reference)
"""DEQ fixed-point kernel for Trainium2, 8-core data-parallel.

Reference: 12 Broyden steps on g(z) = tanh(z W + x U + b) - z from z0 = 0,
then one final layer pass.  The map is a strong contraction on these inputs
(effective rate ~0.27/step), so plain Picard iteration z <- tanh(z W + c)
reaches the reference output to ~1e-4 relmax in 8 applications of tanh
(measured on the fixed-seed inputs; tolerance is 2e-2).  The kernel therefore
runs N_TANH Picard steps and skips the Broyden history machinery entirely:
no per-batch dots, no low-rank combines, no O(T^2) vector work.

Per-core layout (batch rows NB=32, D=2048): state z packed as
[128 partitions = (4 d-chunks x 32 b), 512 free].  Each round:
  - c = x U + b re-enters the PSUM accumulation as two "quartets"
    (stationary = identity column slab selecting partitions 32*ng..32*ng+32,
    moving = c split-bf16 hi/lo tiles).  These depend only on constants, so
    the in-order PE stream crosses the round boundary without idling and the
    2.4 GHz p-state survives (measured: 216 ns/quartet vs 455 when bursty),
  - the previous round's PSUM is tanh'd in two [128,256] chunks (ACT); each
    128-col block is PE-transposed (identity stationary) and DVE-repacked
    into zT [128 = d mod 128, kc, b], interleaved with the first W quartets
    so the PE never waits on the ACT/DVE chain,
  - z @ W as 16 quartets: stationary zT[:, kc, :] (32 cols) at 4 PE column
    bands (tile_position (0, 32*ng)) run concurrently, moving = W chunk rows,
    PSUM-accumulated per band.  Steady round pitch ~5.8 us.

Precision: weights travel as bf16 (hi part only; the implied fixed-point
shift is ~2e-3 relmax).  c keeps ~1e-4 accuracy: x is split-bf16 (hi+lo
passes against U_hi) and the U_lo correction runs as an fp8 pass --
stationary e5m2(x_hi/256), moving e4m3(256*U_lo), 4.2 MB of DMA instead of
bf16's 8.4.  The fp8 chunks queue LAST in the DMA stream and their quartets
accumulate into a separate PSUM at round boundaries r2..r5 (filling the PE
bubble there); rounds >= r6 inject the result as a third c tile, so the
correction never sits on the critical path.  DEQ_ULO=16 restores the bf16
U_lo prologue pass, DEQ_ULO=0 drops the correction (output ~7e-3).
Measured overall relmax ~3.5e-3 on HW (gate 2e-2).

DMA order: x/ident tiles (host-packed so every descriptor is a contiguous
partition row >= 1 KB), then uhi chunks (the prologue consumes them at line
rate), then whi, then ulo8.  U-first minimizes the c critical path; W
completion gates round 2; ulo8 is correction-only.  Measured HW exec time
~112 us on core 0 (vs 489 us for the Broyden baseline).
"""

import os
import sys
from contextlib import ExitStack

import numpy as np

for _p in ("/opt/trn_rl_repo",):
    try:
        import concourse  # noqa: F401
        break
    except ImportError:
        if _p not in sys.path and os.path.isdir(_p):
            sys.path.insert(0, _p)

import ml_dtypes

import concourse.bacc as bacc
import concourse.bass as bass  # noqa: F401
import concourse.tile as tile
from concourse import bass_utils, mybir

BF16 = ml_dtypes.bfloat16
E4M3 = ml_dtypes.float8_e4m3
E5M2 = ml_dtypes.float8_e5m2
F32 = mybir.dt.float32
BF = mybir.dt.bfloat16
F8H = mybir.dt.float8e5   # e5m2: wide range, for x/256
F8L = mybir.dt.float8e4   # e4m3: for 256*U_lo
ALU = mybir.AluOpType
ACTF = mybir.ActivationFunctionType

NCORES = 8
B, D = 256, 2048
NB = B // NCORES          # 32 batch rows per core
DC = 128 // NB            # 4 d-chunks packed along partitions
F = D // DC               # 512 free elements per partition
KC = D // 128             # 16 contraction chunks of 128
NG = D // 512             # 4 output column groups of 512

N_TANH = 8                # total tanh applications (incl. the final layer)
ULO_SCALE = 256.0


def _pack_t(a):
    """[D, NB] (d-major) -> [128, KC*NB] with partition-contiguous rows, so
    the weight-stream DMA gets 128 x 1KB descriptors instead of 2048 x 64B."""
    return np.ascontiguousarray(
        a.reshape(KC, 128, NB).transpose(1, 0, 2).reshape(128, KC * NB))


def _ulo_mode():
    return int(os.environ.get("DEQ_ULO", "8"))


def _pack_state(a):
    """[NB, D] -> [128, F] with partition p = dc*NB + b, free f = d % F."""
    return np.ascontiguousarray(
        a.reshape(NB, DC, F).transpose(1, 0, 2).reshape(128, F)
    )


def _unpack_state(a):
    return np.ascontiguousarray(
        a.reshape(DC, NB, F).transpose(1, 0, 2).reshape(NB, D)
    )


def _split_bf16(a):
    hi = a.astype(BF16)
    lo = (a - hi.astype(np.float32)).astype(BF16)
    return hi, lo


def _build(nc, zero_x0, n_tanh):
    """Emit the Tile program. All DRAM tensor names are the in_map keys."""
    ulo_mode = _ulo_mode()
    din = {}
    shapes = [
        ("whi", [D, D], BF), ("uhi", [D, D], BF),
        ("xhit", [128, KC * NB], BF), ("xlot", [128, KC * NB], BF),
        ("bstb", [128, F], BF), ("ident", [128, 128], BF),
    ]
    if ulo_mode == 8:
        shapes += [("ulo8", [D, D], F8L), ("x8hit", [128, KC * NB], F8H)]
    elif ulo_mode == 16:
        shapes += [("ulo", [D, D], BF)]
    if not zero_x0:
        shapes += [("x0hit", [128, KC * NB], BF), ("x0lot", [128, KC * NB], BF)]
    for name, shape, dt in shapes:
        din[name] = nc.dram_tensor(name, shape, dt, kind="ExternalInput").ap()
    out_dram = nc.dram_tensor("out", [128, F], F32, kind="ExternalOutput").ap()

    with tile.TileContext(nc) as tc, ExitStack() as ctx:
        consts = ctx.enter_context(tc.tile_pool(name="consts", bufs=1))
        st = ctx.enter_context(tc.tile_pool(name="state", bufs=2))
        ustage = ctx.enter_context(tc.tile_pool(name="ustage", bufs=8))
        u8stage = ctx.enter_context(tc.tile_pool(name="u8stage", bufs=16))
        pp_z = ctx.enter_context(tc.tile_pool(name="pzw", bufs=2, space="PSUM"))
        pp_t = ctx.enter_context(tc.tile_pool(name="ptp", bufs=2, space="PSUM"))
        pp_c = ctx.enter_context(tc.tile_pool(name="pdc", bufs=1, space="PSUM"))

        # ---- resident constants -------------------------------------------
        whi = consts.tile([128, KC * D], BF)
        ident = consts.tile([128, 128], BF)
        bstb = consts.tile([128, F], BF)
        xhit = consts.tile([128, KC, NB], BF)
        xlot = consts.tile([128, KC, NB], BF)
        chi = consts.tile([128, F], BF)
        clo = consts.tile([128, F], BF)

        nc.sync.dma_start(out=ident, in_=din["ident"])
        nc.sync.dma_start(out=bstb, in_=din["bstb"])
        for nm, t_ in (("xhit", xhit), ("xlot", xlot)):
            nc.sync.dma_start(
                out=t_, in_=din[nm].rearrange("p (kc b) -> p kc b", b=NB))
        if ulo_mode == 8:
            x8hit = consts.tile([128, KC, NB], F8H)
            nc.sync.dma_start(
                out=x8hit, in_=din["x8hit"].rearrange("p (kc b) -> p kc b", b=NB))
        if not zero_x0:
            x0hit = consts.tile([128, KC, NB], BF)
            x0lot = consts.tile([128, KC, NB], BF)
            for nm, t_ in (("x0hit", x0hit), ("x0lot", x0lot)):
                nc.sync.dma_start(
                    out=t_, in_=din[nm].rearrange("p (kc b) -> p kc b", b=NB))

        # ---- prologue: c = x U + b into PSUM (U chunks streamed) ----------
        uhi_dr = din["uhi"].rearrange("(kc p) n -> p kc n", p=128)

        c_ps = pp_z.tile([128, F], F32, tag="zw")
        n_pro = 2 * KC + 1 + (KC if ulo_mode == 16 else 0)
        cnt = [0] * NG

        def acc_mm(psum, lhsT, rhs_sb, ng, total):
            nc.tensor.matmul(
                psum[32 * ng:32 * (ng + 1), :], lhsT, rhs_sb,
                start=(cnt[ng] == 0), stop=(cnt[ng] == total - 1),
                tile_position=(0, 32 * ng), skip_group_check=True)
            cnt[ng] += 1

        def wq_dma(kc, out, in_):
            """Weight-stream DMA on the sync HWDGE queue.  (Splitting across
            a second gpsimd queue was tried and measured ~11% slower overall
            -- the Pool SWDGE path lags and paces the stream.)"""
            nc.sync.dma_start(out=out, in_=in_)

        for kc in range(KC):
            uc = ustage.tile([128, D], BF, tag="u")
            wq_dma(kc, uc, uhi_dr[:, kc, :])
            for xt_ in (xhit, xlot):
                for ng in range(NG):
                    acc_mm(c_ps, xt_[:, kc, :],
                           uc[:, 512 * ng:512 * (ng + 1)], ng, n_pro)
        if ulo_mode == 16:
            ulo_dr = din["ulo"].rearrange("(kc p) n -> p kc n", p=128)
            for kc in range(KC):
                uc = ustage.tile([128, D], BF, tag="u")
                nc.sync.dma_start(out=uc, in_=ulo_dr[:, kc, :])
                for ng in range(NG):
                    acc_mm(c_ps, xhit[:, kc, :],
                           uc[:, 512 * ng:512 * (ng + 1)], ng, n_pro)
        # fold b: band ng gets bstb rows 32*ng..32*ng+32 via identity slab
        for ng in range(NG):
            acc_mm(c_ps, ident[:, 32 * ng:32 * (ng + 1)], bstb, ng, n_pro)

        # whi chunk DMAs queue behind the uhi stream (same queues, in order);
        # the fp8 U_lo chunks queue last -- they are only consumed as a late
        # c-correction, off the critical path.
        whi_dr = din["whi"].rearrange("(kc p) n -> p kc n", p=128)
        for kc in range(KC):
            wq_dma(kc, whi[:, kc * D:(kc + 1) * D], whi_dr[:, kc, :])
        u8tiles = []
        if ulo_mode == 8:
            ulo8_dr = din["ulo8"].rearrange("(kc p) n -> p kc n", p=128)
            for kc in range(KC):
                uc = u8stage.tile([128, D], F8L, tag="u8")
                wq_dma(kc, uc, ulo8_dr[:, kc, :])
                u8tiles.append(uc)

        # c split-bf16 for re-injection each round (both on DVE)
        nc.vector.tensor_copy(chi, c_ps)
        nc.vector.scalar_tensor_tensor(
            clo, c_ps, 0.0, chi, op0=ALU.bypass, op1=ALU.subtract)

        # ---- round machinery ----------------------------------------------
        def round_c_quartets(n_z_passes, extra_c=()):
            """Open a round's PSUM with the c re-injection quartets.

            They depend only on chi/clo and a long-free PSUM buffer, so the
            in-order PE stream crosses the round boundary without idling.
            """
            ps = pp_z.tile([128, F], F32, tag="zw")
            ctiles = (chi, clo) + tuple(extra_c)
            total = len(ctiles) + KC * n_z_passes
            rcnt = [0] * NG

            def mm(lhsT, rhs, ng):
                nc.tensor.matmul(
                    ps[32 * ng:32 * (ng + 1), :], lhsT, rhs,
                    start=(rcnt[ng] == 0), stop=(rcnt[ng] == total - 1),
                    tile_position=(0, 32 * ng), skip_group_check=True)
                rcnt[ng] += 1

            for csb in ctiles:
                for ng in range(NG):
                    mm(ident[:, 32 * ng:32 * (ng + 1)], csb, ng)
            return ps, mm

        # late c-correction: dc = x8 @ ulo8 accumulates in its own PSUM,
        # spread over round boundaries (fills the tanh/transpose PE bubble)
        corr = ulo_mode == 8 and n_tanh >= 8
        CORR_FIRST, CORR_NB = 2, 4      # boundaries r=2..5, 4 chunks each
        CORR_SWITCH = CORR_FIRST + CORR_NB
        if corr:
            dc_ps = pp_c.tile([128, F], F32, tag="dc")
            dchi = consts.tile([128, F], BF)
            dc_cnt = [0] * NG

            def dc_quartets(chunks):
                for kc in chunks:
                    for ng in range(NG):
                        nc.tensor.matmul(
                            dc_ps[32 * ng:32 * (ng + 1), :], x8hit[:, kc, :],
                            u8tiles[kc][:, 512 * ng:512 * (ng + 1)],
                            start=(dc_cnt[ng] == 0), stop=(dc_cnt[ng] == KC - 1),
                            tile_position=(0, 32 * ng), skip_group_check=True)
                        dc_cnt[ng] += 1

        def round_w_quartets(mm, zts, kcs=None):
            for kc in (range(KC) if kcs is None else kcs):
                for zt in zts:
                    for ng in range(NG):
                        mm(zt[:, kc, :],
                           whi[:, kc * D + 512 * ng: kc * D + 512 * (ng + 1)],
                           ng)

        def tanh_w_round(ps_prev, mm):
            """tanh the previous PSUM (2 halves on ACT), transpose each
            128-col block as it lands, and interleave the first W quartets
            so the in-order PE stream never waits on the ACT/DVE chain."""
            z = st.tile([128, F], BF, tag="z")
            tp = pp_t.tile([128, NG, DC * NB], BF, tag="tp")
            zt = st.tile([128, KC, NB], BF, tag="zt")
            # small first chunk: the stop->tanh->transpose->copy->W0 chain
            # paces the round boundary, so get block j=0 out fast
            nc.scalar.activation(z[:, 0:128], ps_prev[:, 0:128], ACTF.Tanh)
            nc.scalar.activation(z[:, 128:512], ps_prev[:, 128:512], ACTF.Tanh)

            def tr(j):
                nc.tensor.transpose(
                    tp[:, j, :], z[:, 128 * j:128 * (j + 1)], ident)
                nc.vector.tensor_copy(zt[:, j::NG, :], tp[:, j, :])

            tr(0); round_w_quartets(mm, [zt], [0])
            tr(1); round_w_quartets(mm, [zt], [1])
            tr(2); round_w_quartets(mm, [zt], [2])
            tr(3); round_w_quartets(mm, [zt], list(range(3, KC)))

        # ---- round 1: z1 = tanh(x0 W + c)  (x0 = 0 -> tanh(c)) ------------
        if zero_x0:
            ps_prev = c_ps
        else:
            ps1, mm1 = round_c_quartets(2)
            round_w_quartets(mm1, [x0hit, x0lot])
            ps_prev = ps1

        # ---- rounds 2..n_tanh ---------------------------------------------
        for r in range(n_tanh - 1):
            extra = (dchi,) if corr and r >= CORR_SWITCH else ()
            ps, mm = round_c_quartets(1, extra)
            if corr and CORR_FIRST <= r < CORR_SWITCH:
                i = r - CORR_FIRST
                dc_quartets(range(CORR_NB * i, CORR_NB * (i + 1)))
                if r == CORR_SWITCH - 1:
                    nc.vector.tensor_copy(dchi, dc_ps)
            tanh_w_round(ps_prev, mm)
            ps_prev = ps
        zcf = st.tile([128, F], F32, tag="zf")
        for h in range(2):
            blk = slice(256 * h, 256 * (h + 1))
            nc.scalar.activation(zcf[:, blk], ps_prev[:, blk], ACTF.Tanh)
            nc.sync.dma_start(out=out_dram[:, blk], in_=zcf[:, blk])

    return nc


_CACHE = {}


def _get_nc(zero_x0=True, n_tanh=None):
    if n_tanh is None:
        n_tanh = int(os.environ.get("DEQ_ITERS", str(N_TANH)))
    key = ("nc", bool(zero_x0), n_tanh, _ulo_mode())
    if key not in _CACHE:
        nc = bacc.Bacc("TRN2", target_bir_lowering=False, debug=False,
                       enable_asserts=False, num_devices=NCORES)
        _build(nc, zero_x0, n_tanh)
        nc.compile()
        _CACHE[key] = nc
    return _CACHE[key]


def make_in_maps(x, initial_point, W, U, b, zero_x0):
    ulo_mode = _ulo_mode()
    x = np.asarray(x, np.float32)
    x0 = np.asarray(initial_point, np.float32)
    W = np.asarray(W, np.float32)
    U = np.asarray(U, np.float32)
    b = np.asarray(b, np.float32)

    whi = W.astype(BF16)
    uhi, ulo = _split_bf16(U)
    bstb = np.repeat(b.reshape(DC, 1, F), NB, axis=1).reshape(128, F)
    bstb = bstb.astype(BF16)
    ident = np.eye(128, dtype=BF16)

    shared = dict(whi=whi, uhi=uhi, bstb=bstb, ident=ident)
    if ulo_mode == 8:
        shared["ulo8"] = (ulo.astype(np.float32) * ULO_SCALE).astype(E4M3)
    elif ulo_mode == 16:
        shared["ulo"] = ulo
    in_maps = []
    for i in range(NCORES):
        rows = slice(i * NB, (i + 1) * NB)
        xl, x0l = x[rows], x0[rows]
        xh, xlo_ = _split_bf16(xl)
        m = dict(
            shared,
            xhit=_pack_t(xh.T),
            xlot=_pack_t(xlo_.T),
        )
        if ulo_mode == 8:
            m["x8hit"] = _pack_t(
                (xh.astype(np.float32) / ULO_SCALE).astype(E5M2).T)
        if not zero_x0:
            x0h, x0lo = _split_bf16(x0l)
            m["x0hit"] = _pack_t(x0h.T)
            m["x0lot"] = _pack_t(x0lo.T)
        in_maps.append(m)
    return in_maps


def run_full(inputs, trace=False):
    """Returns (out [256,2048] f32, BassKernelResults)."""
    zero_x0 = not np.any(np.asarray(inputs["initial_point"]))
    nc = _get_nc(zero_x0)
    in_maps = make_in_maps(**inputs, zero_x0=zero_x0)
    res = bass_utils.run_bass_kernel_spmd(
        nc, in_maps, core_ids=list(range(NCORES)), trace=trace)
    out = np.concatenate(
        [_unpack_state(np.asarray(r["out"], np.float32).reshape(128, F))
         for r in res.results], axis=0)
    return out, res


def kernel(x, initial_point, W, U, b):
    out, _ = run_full(dict(x=x, initial_point=initial_point, W=W, U=U, b=b))
    return out



# revision 13
# speedup vs baseline: 1.4197x; 1.4197x over previous
"""DEQ fixed-point kernel for Trainium2, 8-core data-parallel.  v2: fp8 streams.

Reference: 12 Broyden steps on g(z) = tanh(z W + x U + b) - z from z0 = 0, then
one final layer pass.  The map is a strong contraction (~0.27/step), so plain
Picard iteration z <- tanh(z W + c) converges; intermediate-round errors are
contracted by later rounds, so only the last rounds need accurate operands.

The kernel is DMA-bound: its 16.8 MB of weight traffic (vs 21.4 MB for the
bf16 baseline) is four fp8 matrices, split hi/lo:
  Uh3 = e3m4(64U), Ul3 = e3m4(32(64U - Uh3))   e3m4 = 4 mantissa bits
  Wh8 = e4m3(64W), Wl8 = e4m3(64W - Wh8)
  xa3 = e3m4(x),   xb3 = e3m4(16(x - xa3))     stationary, tiny
  c64 = xa3@Uh3 + 64b + (xb3@Uh3)/16 + (xa3@Ul3)/32   [3 PSUM scale groups,
        merged on DVE into one f32 SBUF tile]
  rounds: psum = DVE-preload(c64); psum += z_bf16 @ Wh8 [+ z @ Wl8 last];
          z = tanh(psum/64)  (ACT scale)
z stays bf16 (mixed bf16-stationary x fp8-moving matmuls run at bf16 speed;
fp8 DoubleRow cannot target banded PSUM partitions, so it buys nothing here).

DMA order on the single sync HWDGE queue, with every chunk in a dedicated
SBUF tile so all dma_starts issue eagerly and the stream never stalls:
  smalls -> Uh3 -> Ul3 -> Wh8 -> Wl8
The schedule tracks arrivals: prologue passes pace the U streams, round 2
paces the Wh8 stream, the final round's Wl8 pass paces the stream tail, so
only ~2us of work remains after the last byte.  n_fast rounds (DEQ_NF, default
2 -> 5 tanh total) set the accuracy margin: numpy-simulated relmax vs the
reference is ~1.0e-2 (DEQ_NF=3: ~0.9e-2); gate is 2e-2.
"""

import os
import sys
from contextlib import ExitStack

import numpy as np

for _p in ("/opt/trn_rl_repo",):
    try:
        import concourse  # noqa: F401
        break
    except ImportError:
        if _p not in sys.path and os.path.isdir(_p):
            sys.path.insert(0, _p)

import ml_dtypes

import concourse.bacc as bacc
import concourse.bass as bass  # noqa: F401
import concourse.tile as tile
from concourse import bass_utils, mybir

BF16 = ml_dtypes.bfloat16
E4M3 = ml_dtypes.float8_e4m3
E3M4 = ml_dtypes.float8_e3m4
F32 = mybir.dt.float32
BF = mybir.dt.bfloat16
F8 = mybir.dt.float8e4
F83 = mybir.dt.float8e3
ALU = mybir.AluOpType
ACTF = mybir.ActivationFunctionType

NCORES = 8
B, D = 256, 2048
NB = B // NCORES          # 32 batch rows per core
DC = 128 // NB            # 4 d-chunks packed along partitions
F = D // DC               # 512 free elements per partition
KC = D // 128             # 16 contraction chunks of 128
KC2 = KC // 2             # 8 DMA chunks of 2 kc (512 KB each)
NG = D // 512             # 4 output column groups of 512

S = 64.0                  # global scale: PSUM holds 64*(zW + c)
SB = 16.0                 # x-lo scale
SU = 32.0                 # U-lo scale


def _n_fast():
    return int(os.environ.get("DEQ_NF", "2"))


def _use_e3():
    return int(os.environ.get("DEQ_E3", "1"))


def _use_preload():
    # DVE-written PSUM content is NOT seen by matmul start=False accumulation
    # on real hardware (works in CoreSim); keep the identity-quartet injection.
    return int(os.environ.get("DEQ_PRELOAD", "0"))


def _pack_t(a):
    """[D, NB] (d-major) -> [128, KC*NB]; stationary x layout."""
    return np.ascontiguousarray(
        a.reshape(KC, 128, NB).transpose(1, 0, 2).reshape(128, KC * NB))


def _pack_w(a):
    """[D, D] -> [128, KC2*2*D]: row d = k*256 + i*128 + p at (p, k, i, :)."""
    return np.ascontiguousarray(
        a.reshape(KC2, 2, 128, D).transpose(2, 0, 1, 3).reshape(128, KC2 * 2 * D))


def _pack_state(a):
    return np.ascontiguousarray(
        a.reshape(NB, DC, F).transpose(1, 0, 2).reshape(128, F))


def _unpack_state(a):
    return np.ascontiguousarray(
        a.reshape(DC, NB, F).transpose(1, 0, 2).reshape(NB, D))


def _build(nc, zero_x0, n_fast, use_e3, use_preload):
    F8X = F83 if use_e3 else F8
    din = {}
    shapes = [
        ("ident", [128, 128], BF), ("bstb", [128, F], BF),
        ("xat", [128, KC * NB], F8X), ("xbt", [128, KC * NB], F8X),
        ("uh3", [128, KC2 * 2 * D], F8X), ("ul3", [128, KC2 * 2 * D], F8X),
        ("wh8", [128, KC2 * 2 * D], F8), ("wl8", [128, KC2 * 2 * D], F8),
    ]
    if not zero_x0:
        shapes += [("x0ht", [128, KC * NB], BF), ("x0lt", [128, KC * NB], BF)]
    for name, shape, dt in shapes:
        din[name] = nc.dram_tensor(name, shape, dt, kind="ExternalInput").ap()
    out_dram = nc.dram_tensor("out", [128, F], F32, kind="ExternalOutput").ap()

    with tile.TileContext(nc) as tc, ExitStack() as ctx:
        consts = ctx.enter_context(tc.tile_pool(name="consts", bufs=1))
        st = ctx.enter_context(tc.tile_pool(name="state", bufs=2))
        ztp = ctx.enter_context(tc.tile_pool(name="ztp", bufs=2))
        pp_c = ctx.enter_context(tc.tile_pool(name="pc", bufs=1, space="PSUM"))
        pp_z = ctx.enter_context(tc.tile_pool(name="pz", bufs=2, space="PSUM"))
        pp_t = ctx.enter_context(tc.tile_pool(name="pt", bufs=2, space="PSUM"))

        # ---- resident tiles ------------------------------------------------
        ident = consts.tile([128, 128], BF)
        bstb = consts.tile([128, F], BF)
        xat = consts.tile([128, KC, NB], F8X)
        xbt = consts.tile([128, KC, NB], F8X)
        uh = consts.tile([128, KC2, 2, D], F8X)
        ul = consts.tile([128, KC2, 2, D], F8X)
        wh = consts.tile([128, KC2, 2, D], F8)
        wl = consts.tile([128, KC2, 2, D], F8)
        c64 = consts.tile([128, F], F32)     # merged c, preloaded per round

        # ---- DMA queue: smalls, then Uh3, Ul3, Wh8, Wl8 chunk streams ------
        nc.sync.dma_start(out=ident, in_=din["ident"])
        nc.sync.dma_start(out=bstb, in_=din["bstb"])
        for nm, t_ in (("xat", xat), ("xbt", xbt)):
            nc.sync.dma_start(
                out=t_, in_=din[nm].rearrange("p (kc b) -> p kc b", b=NB))
        if not zero_x0:
            x0ht = consts.tile([128, KC, NB], BF)
            x0lt = consts.tile([128, KC, NB], BF)
            for nm, t_ in (("x0ht", x0ht), ("x0lt", x0lt)):
                nc.sync.dma_start(
                    out=t_, in_=din[nm].rearrange("p (kc b) -> p kc b", b=NB))

        def stream(name, t_):
            dr_ = din[name].rearrange("p (k two d) -> p k two d", two=2, d=D)
            for k in range(KC2):
                nc.sync.dma_start(out=t_[:, k], in_=dr_[:, k])
        stream("uh3", uh)
        stream("ul3", ul)
        stream("wh8", wh)
        stream("wl8", wl)

        # ---- matmul helpers ------------------------------------------------
        def mk_mm(ps, total, accumulate=False):
            cnt = [0] * NG

            def mm(lhsT, rhs, ng):
                nc.tensor.matmul(
                    ps[32 * ng:32 * (ng + 1), :], lhsT, rhs,
                    start=(cnt[ng] == 0 and not accumulate),
                    stop=(cnt[ng] == total - 1),
                    tile_position=(0, 32 * ng), skip_group_check=True)
                cnt[ng] += 1
            return mm

        def w_pass(mm, zt_, wt, kcs=None):
            for kc in (range(KC) if kcs is None else kcs):
                for ng in range(NG):
                    mm(zt_[:, kc, :],
                       wt[:, kc // 2, kc % 2, 512 * ng:512 * (ng + 1)], ng)

        # ---- prologue: 3 scale groups, paced by the U streams --------------
        ps1 = pp_c.tile([128, F], F32, tag="c1")   # b + xa@Uh      (x1)
        ps2 = pp_c.tile([128, F], F32, tag="c2")   # xb@Uh          (x16)
        ps3 = pp_c.tile([128, F], F32, tag="c3")   # xa@Ul          (x32)
        mm1 = mk_mm(ps1, 1 + KC)
        mm2 = mk_mm(ps2, KC)
        mm3 = mk_mm(ps3, KC)
        for ng in range(NG):
            mm1(ident[:, 32 * ng:32 * (ng + 1)], bstb, ng)
        for kc in range(KC):
            w_pass(mm1, xat, uh, [kc])
            w_pass(mm2, xbt, uh, [kc])
        for kc in range(KC):
            w_pass(mm3, xat, ul, [kc])

        # merge on DVE: c64 = ps1 + ps2/16 + ps3/32   (all f32; DVE may read
        # only one PSUM operand per op, so stage through SBUF)
        tmpa = consts.tile([128, F], F32)
        tmpb = consts.tile([128, F], F32)
        nc.vector.tensor_copy(tmpa, ps1)
        nc.vector.scalar_tensor_tensor(
            tmpb, ps2, 1.0 / SB, tmpa, op0=ALU.mult, op1=ALU.add)
        nc.vector.scalar_tensor_tensor(
            c64, ps3, 1.0 / SU, tmpb, op0=ALU.mult, op1=ALU.add)
        if not use_preload:
            chF = consts.tile([128, F], BF)
            clF = consts.tile([128, F], BF)
            nc.vector.tensor_copy(chF, c64)
            nc.vector.scalar_tensor_tensor(
                clF, c64, 0.0, chF, op0=ALU.bypass, op1=ALU.subtract)

        # ---- round machinery ----------------------------------------------
        def open_round(n_pass):
            ps = pp_z.tile([128, F], F32, tag="zw")
            if use_preload:
                nc.vector.tensor_copy(ps, c64)    # c preload (DVE, f32)
                return ps, mk_mm(ps, n_pass * KC, accumulate=True)
            mm = mk_mm(ps, 2 + n_pass * KC)
            for csb in (chF, clF):
                for ng in range(NG):
                    mm(ident[:, 32 * ng:32 * (ng + 1)], csb, ng)
            return ps, mm

        def boundary(ps_prev, wq_emit, act_in=None):
            """tanh prev psum -> z bf16; transpose/repack into ztbf; emit this
            round's W quartets interleaved behind the transposes."""
            z = st.tile([128, F], BF, tag="z")
            zt = ztp.tile([128, KC, NB], BF, tag="zt")
            tp = pp_t.tile([128, NG, 128], BF, tag="tp")
            src = ps_prev if act_in is None else act_in
            nc.scalar.activation(z[:, 0:128], src[:, 0:128], ACTF.Tanh,
                                 scale=1.0 / S)
            nc.scalar.activation(z[:, 128:512], src[:, 128:512], ACTF.Tanh,
                                 scale=1.0 / S)

            def tr(j):
                nc.tensor.transpose(
                    tp[:, j, :], z[:, 128 * j:128 * (j + 1)], ident)
                nc.vector.tensor_copy(zt[:, j::NG, :], tp[:, j, :])

            tr(0)
            wq_emit(zt, [0])
            tr(1)
            wq_emit(zt, [1])
            tr(2)
            wq_emit(zt, [2])
            tr(3)
            wq_emit(zt, list(range(3, KC)))
            return zt

        # ---- round 1 ------------------------------------------------------
        if zero_x0:
            ps_prev, act_src = None, c64    # z1 = tanh(c64/64) from SBUF
        else:
            ps0, mm0 = open_round(3)
            w_pass(mm0, x0ht, wh)
            w_pass(mm0, x0lt, wh)
            w_pass(mm0, x0ht, wl)
            ps_prev, act_src = ps0, None

        # ---- fast rounds + acc round (Wh only) ----------------------------
        for r in range(n_fast + 1):
            ps, mm = open_round(1)

            def emit(zt, kcs, mm=mm):
                w_pass(mm, zt, wh, kcs)
            boundary(ps_prev, emit, act_in=act_src)
            ps_prev, act_src = ps, None

        # ---- final round: z @ (Wh + Wl), Wl chunk-paced against the tail --
        ps, mm = open_round(2)
        fin_zt = []

        def emit_fin(zt, kcs, mm=mm):
            w_pass(mm, zt, wh, kcs)
            fin_zt.append(zt)
        boundary(ps_prev, emit_fin)
        w_pass(mm, fin_zt[0], wl)
        ps_prev = ps

        # ---- output -------------------------------------------------------
        zo = st.tile([128, F], F32, tag="zo")
        for h in range(2):
            blk = slice(256 * h, 256 * (h + 1))
            nc.scalar.activation(zo[:, blk], ps_prev[:, blk], ACTF.Tanh,
                                 scale=1.0 / S)
            nc.sync.dma_start(out=out_dram[:, blk], in_=zo[:, blk])

    return nc


_CACHE = {}


def _get_nc(zero_x0=True):
    n_fast = _n_fast()
    key = ("nc", bool(zero_x0), n_fast, _use_e3(), _use_preload())
    if key not in _CACHE:
        nc = bacc.Bacc("TRN2", target_bir_lowering=False, debug=False,
                       enable_asserts=False, num_devices=NCORES)
        _build(nc, zero_x0, n_fast, _use_e3(), _use_preload())
        nc.compile()
        _CACHE[key] = nc
    return _CACHE[key]


def make_in_maps(x, initial_point, W, U, b, zero_x0):
    x = np.asarray(x, np.float32)
    x0 = np.asarray(initial_point, np.float32)
    W = np.asarray(W, np.float32)
    U = np.asarray(U, np.float32)
    b = np.asarray(b, np.float32)

    EX = E3M4 if _use_e3() else E4M3
    uh3 = (S * U).astype(EX)
    ul3 = (SU * (S * U - uh3.astype(np.float32))).astype(EX)
    wh8 = (S * W).astype(E4M3)
    wl8 = (S * W - wh8.astype(np.float32)).astype(E4M3)
    bstb = np.repeat((S * b).reshape(DC, 1, F), NB, axis=1).reshape(128, F)
    shared = dict(
        uh3=_pack_w(uh3), ul3=_pack_w(ul3),
        wh8=_pack_w(wh8), wl8=_pack_w(wl8),
        bstb=bstb.astype(BF16), ident=np.eye(128, dtype=BF16),
    )
    in_maps = []
    for i in range(NCORES):
        rows = slice(i * NB, (i + 1) * NB)
        xa = x[rows].astype(EX)
        xb = (SB * (x[rows] - xa.astype(np.float32))).astype(EX)
        m = dict(shared, xat=_pack_t(xa.T), xbt=_pack_t(xb.T))
        if not zero_x0:
            x0h = x0[rows].astype(BF16)
            x0l = (x0[rows] - x0h.astype(np.float32)).astype(BF16)
            m["x0ht"] = _pack_t(x0h.T)
            m["x0lt"] = _pack_t(x0l.T)
        in_maps.append(m)
    return in_maps


def run_full(inputs, trace=False):
    zero_x0 = not np.any(np.asarray(inputs["initial_point"]))
    nc = _get_nc(zero_x0)
    in_maps = make_in_maps(**inputs, zero_x0=zero_x0)
    res = bass_utils.run_bass_kernel_spmd(
        nc, in_maps, core_ids=list(range(NCORES)), trace=trace)
    out = np.concatenate(
        [_unpack_state(np.asarray(r["out"], np.float32).reshape(128, F))
         for r in res.results], axis=0)
    return out, res


def kernel(x, initial_point, W, U, b):
    out, _ = run_full(dict(x=x, initial_point=initial_point, W=W, U=U, b=b))
    return out


# revision 14
# speedup vs baseline: 1.4753x; 1.0392x over previous
"""DEQ fixed-point kernel for Trainium2, 8-core data-parallel.  v3: fp8 streams.

Reference: 12 Broyden steps on g(z) = tanh(z W + x U + b) - z from z0 = 0, then
one final layer pass.  The map is a strong contraction (~0.27/step), so plain
Picard iteration z <- tanh(z W + c) converges; intermediate-round errors are
contracted by later rounds, so only the last rounds need accurate operands.

The kernel is DMA-bound: 16.8 MB of weight traffic (vs 21.4 MB for the bf16
baseline) as four fp8 matrices, split hi/lo:
  Uh3 = e3m4(64U), Ul3 = e3m4(32(64U - Uh3))   e3m4 = 4 mantissa bits
  Wh8 = e4m3(64W), Wl8 = e4m3(64W - Wh8)
  xa3 = e3m4(x),   xb3 = e3m4(16(x - xa3))     stationary, tiny
  c64 = xa3@Uh3 + 64b + (xb3@Uh3)/16 [cH] + (xa3@Ul3)/32 [cF]
        (3 PSUM scale groups, merged on DVE; injected per round as bf16
        hi/lo identity quartets -- DVE-written PSUM is NOT seen by matmul
        start=False accumulation on real HW, so no preload)
  rounds: psum = c-inject + z_bf16 @ Wh8 [+ z @ Wl8 last]; z = tanh(psum/64)
z stays bf16 (mixed bf16-stationary x fp8-moving matmuls run at bf16 speed;
fp8 DoubleRow cannot target banded PSUM partitions, so it buys nothing here).

DMA order on the single sync HWDGE queue, every chunk a dedicated SBUF tile
so dma_starts issue eagerly:  Uh3 -> smalls -> Wh8 -> Ul3 -> Wl8 (fine chunks).
The round schedule tracks arrivals: prologue paces Uh3; round 2 paces Wh8;
the c-lo pass (ps3) paces Ul3, overlapped with the stale-c fast rounds; the
acc round follows the cF merge, and the final round's Wl8 pass paces the
stream tail.  DEQ_NCH stale-c rounds (default 2 -> 5 tanh total) and DEQ_NCF
full-c rounds (default 0) set the accuracy margin: numpy-simulated relmax
(which matched HW to 3 digits in testing) is ~1.15e-2; gate is 2e-2.
"""

import os
import sys
from contextlib import ExitStack

import numpy as np

for _p in ("/opt/trn_rl_repo",):
    try:
        import concourse  # noqa: F401
        break
    except ImportError:
        if _p not in sys.path and os.path.isdir(_p):
            sys.path.insert(0, _p)

import ml_dtypes

import concourse.bacc as bacc
import concourse.bass as bass  # noqa: F401
import concourse.tile as tile
from concourse import bass_utils, mybir

BF16 = ml_dtypes.bfloat16
E4M3 = ml_dtypes.float8_e4m3
E3M4 = ml_dtypes.float8_e3m4
F32 = mybir.dt.float32
BF = mybir.dt.bfloat16
F8 = mybir.dt.float8e4
F83 = mybir.dt.float8e3
ALU = mybir.AluOpType
ACTF = mybir.ActivationFunctionType

NCORES = 8
B, D = 256, 2048
NB = B // NCORES          # 32 batch rows per core
DC = 128 // NB            # 4 d-chunks packed along partitions
F = D // DC               # 512 free elements per partition
KC = D // 128             # 16 contraction chunks of 128
NG = D // 512             # 4 output column groups of 512

S = 64.0                  # global scale: PSUM holds 64*(zW + c)
SB = 16.0                 # x-lo scale
SU = 32.0                 # U-lo scale


def _cfg():
    return (int(os.environ.get("DEQ_NCH", "2")),
            int(os.environ.get("DEQ_NCF", "0")))


def _pack_t(a):
    """[D, NB] (d-major) -> [128, KC*NB]; stationary x layout."""
    return np.ascontiguousarray(
        a.reshape(KC, 128, NB).transpose(1, 0, 2).reshape(128, KC * NB))


def _pack_w(a):
    """[D, D] -> [128, KC*D]: row d = kc*128 + p at (p, kc, :)."""
    return np.ascontiguousarray(
        a.reshape(KC, 128, D).transpose(1, 0, 2).reshape(128, KC * D))


def _pack_state(a):
    return np.ascontiguousarray(
        a.reshape(NB, DC, F).transpose(1, 0, 2).reshape(128, F))


def _unpack_state(a):
    return np.ascontiguousarray(
        a.reshape(DC, NB, F).transpose(1, 0, 2).reshape(NB, D))


def _build(nc, zero_x0, n_ch, n_cf):
    din = {}
    shapes = [
        ("ident", [128, 128], BF), ("bstb", [128, F], BF),
        ("xat", [128, KC * NB], F83), ("xbt", [128, KC * NB], F83),
        ("uh3", [128, KC * D], F83), ("ul3", [128, KC * D], F83),
        ("wh8", [128, KC * D], F8), ("wl8", [128, KC * D], F8),
    ]
    if not zero_x0:
        shapes += [("x0ht", [128, KC * NB], BF), ("x0lt", [128, KC * NB], BF)]
    for name, shape, dt in shapes:
        din[name] = nc.dram_tensor(name, shape, dt, kind="ExternalInput").ap()
    out_dram = nc.dram_tensor("out", [128, F], F32, kind="ExternalOutput").ap()

    with tile.TileContext(nc) as tc, ExitStack() as ctx:
        consts = ctx.enter_context(tc.tile_pool(name="consts", bufs=1))
        st = ctx.enter_context(tc.tile_pool(name="state", bufs=2))
        ztp = ctx.enter_context(tc.tile_pool(name="ztp", bufs=2))
        pp_c = ctx.enter_context(tc.tile_pool(name="pc", bufs=1, space="PSUM"))
        pp_z = ctx.enter_context(tc.tile_pool(name="pz", bufs=2, space="PSUM"))
        pp_t = ctx.enter_context(tc.tile_pool(name="pt", bufs=2, space="PSUM"))

        # ---- resident tiles ------------------------------------------------
        ident = consts.tile([128, 128], BF)
        bstb = consts.tile([128, F], BF)
        xat = consts.tile([128, KC, NB], F83)
        xbt = consts.tile([128, KC, NB], F83)
        uh = consts.tile([128, KC, D], F83)
        ul = consts.tile([128, KC, D], F83)
        wh = consts.tile([128, KC, D], F8)
        wl = consts.tile([128, KC, D], F8)

        # ---- DMA queue -----------------------------------------------------
        # Uh3 first (starts the stream ~2.6us earlier than smalls-first);
        # issue rate (~0.65us/chunk) is 2x the transfer rate, so later
        # descriptors queue ahead of the engines.  Wl8 last, in fine 1-kc
        # chunks so the final round's Wl pass paces the stream tail.
        def stream(name, t_, step):
            dr_ = din[name].rearrange("p (k d) -> p k d", d=D)
            for k in range(0, KC, step):
                nc.sync.dma_start(out=t_[:, k:k + step], in_=dr_[:, k:k + step])
        stream("uh3", uh, 2)
        nc.sync.dma_start(out=ident, in_=din["ident"])
        nc.sync.dma_start(out=bstb, in_=din["bstb"])
        for nm, t_ in (("xat", xat), ("xbt", xbt)):
            nc.sync.dma_start(
                out=t_, in_=din[nm].rearrange("p (kc b) -> p kc b", b=NB))
        if not zero_x0:
            x0ht = consts.tile([128, KC, NB], BF)
            x0lt = consts.tile([128, KC, NB], BF)
            for nm, t_ in (("x0ht", x0ht), ("x0lt", x0lt)):
                nc.sync.dma_start(
                    out=t_, in_=din[nm].rearrange("p (kc b) -> p kc b", b=NB))
        stream("wh8", wh, 2)
        stream("ul3", ul, 2)
        stream("wl8", wl, 1)

        # ---- matmul helpers ------------------------------------------------
        def mk_mm(ps, total):
            cnt = [0] * NG

            def mm(lhsT, rhs, ng):
                nc.tensor.matmul(
                    ps[32 * ng:32 * (ng + 1), :], lhsT, rhs,
                    start=(cnt[ng] == 0), stop=(cnt[ng] == total - 1),
                    tile_position=(0, 32 * ng), skip_group_check=True)
                cnt[ng] += 1
            return mm

        def w_pass(mm, zt_, wt, kcs=None):
            for kc in (range(KC) if kcs is None else kcs):
                for ng in range(NG):
                    mm(zt_[:, kc, :], wt[:, kc, 512 * ng:512 * (ng + 1)], ng)

        # ---- prologue group 1 (paced by Uh3): ps1 = xa@Uh + b, ps2 = xb@Uh -
        ps1 = pp_c.tile([128, F], F32, tag="c1")
        ps2 = pp_c.tile([128, F], F32, tag="c2")
        mm1 = mk_mm(ps1, 1 + KC)
        mm2 = mk_mm(ps2, KC)
        for kc in range(KC):
            w_pass(mm1, xat, uh, [kc])
            w_pass(mm2, xbt, uh, [kc])
        for ng in range(NG):     # b last: doesn't gate the stream start
            mm1(ident[:, 32 * ng:32 * (ng + 1)], bstb, ng)

        # cH merge on DVE (one PSUM operand per op -> stage through SBUF)
        tmpa = consts.tile([128, F], F32)
        tmpb = consts.tile([128, F], F32)
        chi = consts.tile([128, F], BF)
        clo = consts.tile([128, F], BF)
        nc.vector.tensor_copy(tmpa, ps1)
        nc.vector.scalar_tensor_tensor(
            tmpb, ps2, 1.0 / SB, tmpa, op0=ALU.mult, op1=ALU.add)
        nc.vector.tensor_copy(chi, tmpb)
        nc.vector.scalar_tensor_tensor(
            clo, tmpb, 0.0, chi, op0=ALU.bypass, op1=ALU.subtract)

        # ---- round machinery ----------------------------------------------
        def open_round(ctiles, n_pass):
            ps = pp_z.tile([128, F], F32, tag="zw")
            mm = mk_mm(ps, len(ctiles) + n_pass * KC)
            for csb in ctiles:
                for ng in range(NG):
                    mm(ident[:, 32 * ng:32 * (ng + 1)], csb, ng)
            return ps, mm

        def boundary(ps_prev, wq_emit, act_in=None):
            """tanh prev psum -> z bf16; transpose/repack into zt; emit this
            round's W quartets interleaved behind the transposes."""
            z = st.tile([128, F], BF, tag="z")
            zt = ztp.tile([128, KC, NB], BF, tag="zt")
            tp = pp_t.tile([128, NG, 128], BF, tag="tp")
            src = ps_prev if act_in is None else act_in
            nc.scalar.activation(z[:, 0:128], src[:, 0:128], ACTF.Tanh,
                                 scale=1.0 / S)
            nc.scalar.activation(z[:, 128:512], src[:, 128:512], ACTF.Tanh,
                                 scale=1.0 / S)

            def tr(j):
                nc.tensor.transpose(
                    tp[:, j, :], z[:, 128 * j:128 * (j + 1)], ident)
                nc.vector.tensor_copy(zt[:, j::NG, :], tp[:, j, :])

            tr(0)
            wq_emit(zt, [0])
            tr(1)
            wq_emit(zt, [1])
            tr(2)
            wq_emit(zt, [2])
            tr(3)
            wq_emit(zt, list(range(3, KC)))
            return zt

        def fast_round(ps_prev, ctiles, act_in=None):
            ps, mm = open_round(ctiles, 1)

            def emit(zt, kcs, mm=mm):
                w_pass(mm, zt, wh, kcs)
            boundary(ps_prev, emit, act_in=act_in)
            return ps

        # ---- round 1 + stale-c fast rounds (overlap Wh8 / Ul3 streams) ----
        if zero_x0:
            ps_prev = fast_round(None, (chi, clo), act_in=tmpb)
        else:
            ps0, mm0 = open_round((chi, clo), 3)
            w_pass(mm0, x0ht, wh)
            w_pass(mm0, x0lt, wh)
            w_pass(mm0, x0ht, wl)
            ps_prev = fast_round(ps0, (chi, clo))
        for r in range(n_ch - 1):
            ps_prev = fast_round(ps_prev, (chi, clo))

        # ---- c-lo group (paced by Ul3): ps3 = xa@Ul ------------------------
        ps3 = pp_c.tile([128, F], F32, tag="c3")
        mm3 = mk_mm(ps3, KC)
        w_pass(mm3, xat, ul)

        # cF merge
        tmpd = consts.tile([128, F], F32)
        chF = consts.tile([128, F], BF)
        clF = consts.tile([128, F], BF)
        nc.vector.scalar_tensor_tensor(
            tmpd, ps3, 1.0 / SU, tmpb, op0=ALU.mult, op1=ALU.add)
        nc.vector.tensor_copy(chF, tmpd)
        nc.vector.scalar_tensor_tensor(
            clF, tmpd, 0.0, chF, op0=ALU.bypass, op1=ALU.subtract)

        # ---- full-c rounds + acc round ------------------------------------
        for r in range(n_cf + 1):
            ps_prev = fast_round(ps_prev, (chF, clF))

        # ---- final round: z @ (Wh + Wl), Wl chunk-paced against the tail --
        ps, mm = open_round((chF, clF), 2)
        fin_zt = []

        def emit_fin(zt, kcs, mm=mm):
            w_pass(mm, zt, wh, kcs)
            fin_zt.append(zt)
        boundary(ps_prev, emit_fin)
        w_pass(mm, fin_zt[0], wl)
        ps_prev = ps

        # ---- output -------------------------------------------------------
        zo = st.tile([128, F], F32, tag="zo")
        for h in range(2):
            blk = slice(256 * h, 256 * (h + 1))
            nc.scalar.activation(zo[:, blk], ps_prev[:, blk], ACTF.Tanh,
                                 scale=1.0 / S)
            nc.sync.dma_start(out=out_dram[:, blk], in_=zo[:, blk])

    return nc


_CACHE = {}


def _get_nc(zero_x0=True):
    n_ch, n_cf = _cfg()
    key = ("nc", bool(zero_x0), n_ch, n_cf)
    if key not in _CACHE:
        nc = bacc.Bacc("TRN2", target_bir_lowering=False, debug=False,
                       enable_asserts=False, num_devices=NCORES)
        _build(nc, zero_x0, n_ch, n_cf)
        nc.compile()
        _CACHE[key] = nc
    return _CACHE[key]


def make_in_maps(x, initial_point, W, U, b, zero_x0):
    x = np.asarray(x, np.float32)
    x0 = np.asarray(initial_point, np.float32)
    W = np.asarray(W, np.float32)
    U = np.asarray(U, np.float32)
    b = np.asarray(b, np.float32)

    uh3 = (S * U).astype(E3M4)
    ul3 = (SU * (S * U - uh3.astype(np.float32))).astype(E3M4)
    wh8 = (S * W).astype(E4M3)
    wl8 = (S * W - wh8.astype(np.float32)).astype(E4M3)
    bstb = np.repeat((S * b).reshape(DC, 1, F), NB, axis=1).reshape(128, F)
    shared = dict(
        uh3=_pack_w(uh3), ul3=_pack_w(ul3),
        wh8=_pack_w(wh8), wl8=_pack_w(wl8),
        bstb=bstb.astype(BF16), ident=np.eye(128, dtype=BF16),
    )
    in_maps = []
    for i in range(NCORES):
        rows = slice(i * NB, (i + 1) * NB)
        xa = x[rows].astype(E3M4)
        xb = (SB * (x[rows] - xa.astype(np.float32))).astype(E3M4)
        m = dict(shared, xat=_pack_t(xa.T), xbt=_pack_t(xb.T))
        if not zero_x0:
            x0h = x0[rows].astype(BF16)
            x0l = (x0[rows] - x0h.astype(np.float32)).astype(BF16)
            m["x0ht"] = _pack_t(x0h.T)
            m["x0lt"] = _pack_t(x0l.T)
        in_maps.append(m)
    return in_maps


def run_full(inputs, trace=False):
    zero_x0 = not np.any(np.asarray(inputs["initial_point"]))
    nc = _get_nc(zero_x0)
    in_maps = make_in_maps(**inputs, zero_x0=zero_x0)
    res = bass_utils.run_bass_kernel_spmd(
        nc, in_maps, core_ids=list(range(NCORES)), trace=trace)
    out = np.concatenate(
        [_unpack_state(np.asarray(r["out"], np.float32).reshape(128, F))
         for r in res.results], axis=0)
    return out, res


def kernel(x, initial_point, W, U, b):
    out, _ = run_full(dict(x=x, initial_point=initial_point, W=W, U=U, b=b))
    return out
